# revision 1
# baseline (speedup 1.0000x reference)
"""v3: feature-major LSTM cell kernel, host-relayout + layout-L gates.

Host prep per shard (part of the sharding strategy):
  xh12 [98, R] bf16 : rows 0:49 = A1 = bf16([x|h|ones].T), rows 49:98 = A2 = bf16(A - A1)
  cT   [32, R] f32  : c.T
  w_even [128, G] bf16 : rows 0:49 = W1 = bf16(W_aug), rows 64:113 = W1 again
  w_odd  [49, G] bf16  : W2 = bf16(W_aug - W1)
Outputs hT,cT_new [32, R] f32 are transposed back on host.

Device, per 2048-row group (4 chunks x 512):
  - DMA A1 -> sbuf parts 0:49, A2 -> parts 64:113; cT -> layout-L [128, 512]
    (partition p = 32*q + hdim, q = chunk index)
  - 48 matmuls bf16 (4 gates x 4 chunks x 3 terms), tile_position col-packed,
    accumulating into IFO_ps [128,3,512] and G_ps [128,512] (layout-L)
  - ACT: sigmoid(IFO) [128,3,512], tanh(G); DVE: m1=I*G, m2=F*C, cn=m1+m2;
    ACT: tc=tanh(cn); DVE: hn=O*tc   (all full-lane [128,512] ops)
  - DMA out cn, hn -> cT_new/hT feature-major
"""

import sys

if "/opt/trn_rl_repo" not in sys.path:
    sys.path.insert(0, "/opt/trn_rl_repo")

import ml_dtypes
import numpy as np

import bass_rust
import concourse.bass as bass
import concourse.tile as tile
from concourse import mybir

F32 = mybir.dt.float32
BF16 = mybir.dt.bfloat16
AF = mybir.ActivationFunctionType

B = 1048576
N_CORES = 8
R = B // N_CORES
IN_DIM, H_DIM = 16, 32
XH = IN_DIM + H_DIM
K_AUG = XH + 1  # 49
G4 = 4 * H_DIM  # 128
P = 128
TF = 512  # rows per chunk (matmul free dim)
NQ = 4  # chunks per group
GRP = NQ * TF  # 2048 rows per group

# gate -> (dest, sub) where dest 0 = IFO psum tile slot index, -1 = G tile
GATE_COLS = {"i": (0, 32), "f": (32, 64), "g": (64, 96), "o": (96, 128)}


def _split_waits(nc, max_waits=1):
    """Walrus codegen allows at most one semaphore wait per instruction.

    Move excess waits onto preceding same-engine EventSemaphore (pure wait)
    instructions; program order on the engine queue makes this equivalent.
    """
    n = 0
    for f in nc.m.functions:
        for blk in f.blocks:
            insts = blk.instructions
            new = []
            for inst in insts:
                si = inst.sync_info
                waits = list(si.on_wait) if si and si.on_wait else []
                if len(waits) > max_waits:
                    excess, keep = waits[:-max_waits], waits[-max_waits:]
                    for j in range(0, len(excess), max_waits):
                        nop = mybir.InstEventSemaphore(
                            name=f"{inst.name}-tw{j}", ins=[], outs=[]
                        )
                        nop.engine = inst.engine
                        nop.sync_info = bass_rust.SyncInfo(
                            on_wait=excess[j : j + max_waits], on_update=[]
                        )
                        new.append(nop)
                        n += 1
                    si.on_wait = keep
                    inst.sync_info = si
                new.append(inst)
            insts[:] = new
    return n


def build_nc(rows=R, split_waits=True, repeat=1, dma_mode="swdge3d", terms=2, tail=True):
    assert rows % GRP == 0
    ngrp = rows // GRP

    nc = bass.Bass()
    xh12 = nc.dram_tensor("xh12", [2 * K_AUG, rows], BF16, kind="ExternalInput")
    cT = nc.dram_tensor("cT", [H_DIM, rows], F32, kind="ExternalInput")
    w1 = nc.dram_tensor("w1", [2 * K_AUG, G4], BF16, kind="ExternalInput")
    w2 = nc.dram_tensor("w2", [2 * K_AUG, G4], BF16, kind="ExternalInput")
    hT = nc.dram_tensor("hT", [H_DIM, rows], F32, kind="ExternalOutput")
    cTn = nc.dram_tensor("cTn", [H_DIM, rows], F32, kind="ExternalOutput")

    with tile.TileContext(nc) as tc:
        with (
            tc.tile_pool(name="const", bufs=1) as constp,
            tc.tile_pool(name="io", bufs=3) as iop,
            tc.tile_pool(name="work", bufs=3) as workp,
            tc.tile_pool(name="psum", bufs=2, space="PSUM") as psump,
        ):
            w1_sb = constp.tile([2 * K_AUG, G4], BF16, tag="w1")
            nc.sync.dma_start(w1_sb[:], w1[:])
            w2_sb = constp.tile([2 * K_AUG, G4], BF16, tag="w2")
            nc.sync.dma_start(w2_sb[:], w2[:])

            import contextlib

            rep_ctx = tc.For_i(0, repeat, 1) if repeat > 1 else contextlib.nullcontext()
            with rep_ctx:
              for it in range(ngrp):
                  off = it * GRP
                  xh_sb = iop.tile([2 * K_AUG, GRP], BF16, tag="xh")
                  nc.sync.dma_start(xh_sb[:], xh12[:, off : off + GRP])
                  # layout-L c: partition 32q+h <- cT[h, off + q*TF + t]
                  c_sb = iop.tile([P, TF], F32, tag="c")
                  if not tail:
                      pass
                  elif dma_mode == "swdge3d":
                      cin = cT[:, off : off + GRP].rearrange("h (q t) -> q h t", q=NQ)
                      nc.gpsimd.dma_start(c_sb[:], cin)
                  elif dma_mode == "hwdge3d":
                      cin = cT[:, off : off + GRP].rearrange("h (q t) -> q h t", q=NQ)
                      nc.scalar.dma_start(c_sb[:], cin)
                  else:
                      for q in range(NQ):
                          nc.sync.dma_start(
                              c_sb[32 * q : 32 * q + 32, :],
                              cT[:, off + q * TF : off + (q + 1) * TF],
                          )

                  ifo_ps = psump.tile([P, 3, TF], F32, tag="ifo")
                  g_ps = psump.tile([P, TF], F32, tag="g")

                  def dest_ap(gate, q):
                      if gate == "i":
                          return ifo_ps[32 * q : 32 * q + 32, 0, :]
                      if gate == "f":
                          return ifo_ps[32 * q : 32 * q + 32, 1, :]
                      if gate == "o":
                          return ifo_ps[32 * q : 32 * q + 32, 2, :]
                      return g_ps[32 * q : 32 * q + 32, :]

                  for gate in ("i", "f", "g", "o"):
                      c0, c1 = GATE_COLS[gate]
                      # per column-strip q: K=98 stacked [A1;A2] against
                      # [W1;W1] then [W2;W2] -> A@(W1+W2), full split
                      # precision in 2 matmuls. Both at array row 0 (mixing
                      # row positions in one accum group faults the HW).
                      for q in range(NQ):
                          rhs = xh_sb[:, bass.ts(q, TF)]
                          nc.tensor.matmul(
                              dest_ap(gate, q),
                              w1_sb[:, c0:c1],
                              rhs,
                              start=True,
                              stop=(terms == 1),
                              tile_position=(0, 32 * q),
                          )
                          if terms == 2:
                              nc.tensor.matmul(
                                  dest_ap(gate, q),
                                  w2_sb[:, c0:c1],
                                  rhs,
                                  start=False,
                                  stop=True,
                                  tile_position=(0, 32 * q),
                              )

                  if not tail:
                      small = workp.tile([P, 3, 8], F32, tag="small")
                      nc.scalar.activation(small[:], ifo_ps[:, :, 0:8], AF.Sigmoid)
                      nc.vector.tensor_copy(small[:, 0, :], g_ps[:, 0:8])
                      nc.sync.dma_start(hT[:, off : off + 8], small[0:32, 0, :])
                      continue
                  ifo_sb = workp.tile([P, 3, TF], F32, tag="ifo_sb")
                  nc.scalar.activation(ifo_sb[:], ifo_ps[:], AF.Sigmoid)
                  g_sb = workp.tile([P, TF], F32, tag="g_sb")
                  nc.scalar.activation(g_sb[:], g_ps[:], AF.Tanh)

                  m1 = workp.tile([P, TF], F32, tag="m1")
                  nc.vector.tensor_mul(m1[:], ifo_sb[:, 0, :], g_sb[:])
                  m2 = workp.tile([P, TF], F32, tag="m2")
                  nc.vector.tensor_mul(m2[:], ifo_sb[:, 1, :], c_sb[:])
                  cn = workp.tile([P, TF], F32, tag="cn")
                  nc.vector.tensor_add(cn[:], m1[:], m2[:])
                  tc_sb = workp.tile([P, TF], F32, tag="tc")
                  nc.scalar.activation(tc_sb[:], cn[:], AF.Tanh)
                  hn = workp.tile([P, TF], F32, tag="hn")
                  nc.vector.tensor_mul(hn[:], ifo_sb[:, 2, :], tc_sb[:])

                  cout = cTn[:, off : off + GRP].rearrange("h (q t) -> q h t", q=NQ)
                  hout = hT[:, off : off + GRP].rearrange("h (q t) -> q h t", q=NQ)
                  if dma_mode == "swdge3d":
                      nc.gpsimd.dma_start(cout, cn[:])
                      nc.gpsimd.dma_start(hout, hn[:])
                  elif dma_mode == "hwdge3d":
                      nc.scalar.dma_start(cout, cn[:])
                      nc.scalar.dma_start(hout, hn[:])
                  else:
                      for q in range(NQ):
                          nc.sync.dma_start(
                              cTn[:, off + q * TF : off + (q + 1) * TF],
                              cn[32 * q : 32 * q + 32, :],
                          )
                          nc.sync.dma_start(
                              hT[:, off + q * TF : off + (q + 1) * TF],
                              hn[32 * q : 32 * q + 32, :],
                          )

    if split_waits:
        _split_waits(nc)
    return nc


def host_prep(x, h, c, Wx, Wh, b):
    """Build per-full-batch host arrays (sharding slices columns)."""
    n = x.shape[0]
    A = np.empty((K_AUG, n), dtype=np.float32)
    A[0:IN_DIM] = np.asarray(x, np.float32).T
    A[IN_DIM:XH] = np.asarray(h, np.float32).T
    A[XH] = 1.0
    A1 = A.astype(ml_dtypes.bfloat16)
    A2 = (A - A1.astype(np.float32)).astype(ml_dtypes.bfloat16)
    xh12 = np.concatenate([A1, A2], axis=0)  # [98, n] bf16

    W = np.concatenate(
        [np.asarray(Wx), np.asarray(Wh), np.asarray(b)[None, :]], axis=0
    ).astype(np.float32)  # [49, 128]
    W1s = W.astype(ml_dtypes.bfloat16)
    W2s = (W - W1s.astype(np.float32)).astype(ml_dtypes.bfloat16)
    W1 = np.ascontiguousarray(np.concatenate([W1s, W1s], axis=0))
    W2 = np.ascontiguousarray(np.concatenate([W2s, W2s], axis=0))

    cTfull = np.ascontiguousarray(np.asarray(c, np.float32).T)  # [32, n]
    return xh12, cTfull, W1, W2


_NC_CACHE = {}


def _get_nc(rows=R):
    if rows not in _NC_CACHE:
        _NC_CACHE[rows] = build_nc(rows)
    return _NC_CACHE[rows]


def run(x, h, c, Wx, Wh, b, trace=False, rows=R, n_cores=N_CORES):
    """Shard, execute on the 8 cores, gather. Returns (h_new, c_new, results)."""
    from concourse.bass_utils import run_bass_kernel_spmd

    xh12, cTfull, w1_np, w2_np = host_prep(x, h, c, Wx, Wh, b)
    nc = _get_nc(rows)
    in_maps = []
    for i in range(n_cores):
        sl = slice(i * rows, (i + 1) * rows)
        in_maps.append(
            {
                "xh12": np.ascontiguousarray(xh12[:, sl]),
                "cT": np.ascontiguousarray(cTfull[:, sl]),
                "w1": w1_np,
                "w2": w2_np,
            }
        )
    res = run_bass_kernel_spmd(nc, in_maps, list(range(n_cores)), trace=trace)
    n = rows * n_cores
    h_new = np.empty((n, H_DIM), dtype=np.float32)
    c_new = np.empty((n, H_DIM), dtype=np.float32)
    for i, r in enumerate(res.results):
        sl = slice(i * rows, (i + 1) * rows)
        h_new[sl] = r["hT"].T
        c_new[sl] = r["cTn"].T
    return h_new, c_new, res


def kernel(x, h, c, Wx, Wh, b):
    h_new, c_new, _ = run(x, h, c, Wx, Wh, b)
    return h_new, c_new



# revision 3
# speedup vs baseline: 2.1063x; 2.1063x over previous
"""v4: row-major LSTM cell kernel, fp16 I/O, full-lane elementwise.

Sharding: pure data-parallel, batch split 8 ways (131072 rows/core).

Host prep (layout only; all compute on device):
  W_aug [49,128] fp16: rows = [Wx; Wh; b], gate cols reordered [i|f|o|g]
  xh    [nwin, 49, 2048] fp16: window w, col m*128+p = [x|h|1](row w*2048+16p+m)
  c     [nwin, 128, 16, 32] fp16 = natural reshape (row = w*2048+16p+m)
Outputs hn, cn [nwin, 128, 16, 32] fp16 -> natural reshape back, cast f32.

Device, per 2048-row window:
  - 16 matmuls: stationary lhsT = xh[:, 128m:128m+128] (49x128 rows-chunk),
    moving rhs = W [49,128] -> PSUM [128 rows, 16 chunks, 128 gates] f32
    (4 banks; partition = row, free = gate -> row-major)
  - ACT: sigmoid PSUM[:,:,0:96] -> sfo fp16; tanh PSUM[:,:,96:128] -> g fp16
  - DVE (all fp16 SBUF, 2x mode): m1=i*g, m2=f*c, cn=m1+m2, hn=o*tanh(cn)
  - ACT: tc=tanh(cn)
  - DMA out cn, hn as [128, 1KB] contiguous blocks
"""

import sys

if "/opt/trn_rl_repo" not in sys.path:
    sys.path.insert(0, "/opt/trn_rl_repo")

import ml_dtypes
import numpy as np

import bass_rust
import concourse.bass as bass
import concourse.tile as tile
from concourse import mybir

F32 = mybir.dt.float32
F16 = mybir.dt.float16
AF = mybir.ActivationFunctionType

B = 1048576
N_CORES = 8
R = B // N_CORES
IN_DIM, H_DIM = 16, 32
XH = IN_DIM + H_DIM
K_AUG = XH + 1  # 49
G4 = 4 * H_DIM  # 128
P = 128
CH = 16  # chunks (of 128 rows) per window
WIN = CH * P  # 2048 rows per window
NWIN = R // WIN  # 64


def _split_waits(nc, max_waits=1):
    """Walrus codegen allows at most one semaphore wait per instruction."""
    n = 0
    for f in nc.m.functions:
        for blk in f.blocks:
            insts = blk.instructions
            new = []
            for inst in insts:
                si = inst.sync_info
                waits = list(si.on_wait) if si and si.on_wait else []
                if len(waits) > max_waits:
                    excess, keep = waits[:-max_waits], waits[-max_waits:]
                    for j in range(0, len(excess), max_waits):
                        nop = mybir.InstEventSemaphore(
                            name=f"{inst.name}-tw{j}", ins=[], outs=[]
                        )
                        nop.engine = inst.engine
                        nop.sync_info = bass_rust.SyncInfo(
                            on_wait=excess[j : j + max_waits], on_update=[]
                        )
                        new.append(nop)
                        n += 1
                    si.on_wait = keep
                    inst.sync_info = si
                new.append(inst)
            insts[:] = new
    return n


def build_nc(rows=R):
    assert rows % WIN == 0
    nwin = rows // WIN

    nc = bass.Bass()
    xh = nc.dram_tensor("xh", [nwin, K_AUG, WIN], F16, kind="ExternalInput")
    c_in = nc.dram_tensor("c_in", [nwin, P, CH * H_DIM], F16, kind="ExternalInput")
    w = nc.dram_tensor("w", [K_AUG, G4], F16, kind="ExternalInput")
    hn_out = nc.dram_tensor("hn", [nwin, P, CH * H_DIM], F16, kind="ExternalOutput")
    cn_out = nc.dram_tensor("cn", [nwin, P, CH * H_DIM], F16, kind="ExternalOutput")

    with tile.TileContext(nc) as tc:
        with (
            tc.tile_pool(name="const", bufs=1) as constp,
            tc.tile_pool(name="io", bufs=3) as iop,
            tc.tile_pool(name="work", bufs=3) as workp,
            tc.tile_pool(name="psum", bufs=2, space="PSUM") as psump,
        ):
            w_sb = constp.tile([K_AUG, G4], F16, tag="w")
            nc.sync.dma_start(w_sb[:], w[:])

            for it in range(nwin):
                xh_sb = iop.tile([K_AUG, WIN], F16, tag="xh")
                nc.sync.dma_start(xh_sb[:], xh[it])
                c_sb = iop.tile([P, CH, H_DIM], F16, tag="c")
                nc.sync.dma_start(c_sb[:], c_in[it])

                ps = psump.tile([P, CH, G4], F32, tag="ps")
                for m in range(CH):
                    nc.tensor.matmul(
                        ps[:, m, :],
                        xh_sb[:, bass.ts(m, P)],
                        w_sb[:],
                        start=True,
                        stop=True,
                    )

                sfo = workp.tile([P, CH, 3 * H_DIM], F16, tag="sfo")
                nc.scalar.activation(sfo[:], ps[:, :, 0 : 3 * H_DIM], AF.Sigmoid)
                g_sb = workp.tile([P, CH, H_DIM], F16, tag="g")
                nc.scalar.activation(g_sb[:], ps[:, :, 3 * H_DIM : G4], AF.Tanh)

                m1 = workp.tile([P, CH, H_DIM], F16, tag="m1")
                nc.vector.tensor_mul(m1[:], sfo[:, :, 0:H_DIM], g_sb[:])
                m2 = workp.tile([P, CH, H_DIM], F16, tag="m2")
                nc.vector.tensor_mul(m2[:], sfo[:, :, H_DIM : 2 * H_DIM], c_sb[:])
                cn = workp.tile([P, CH, H_DIM], F16, tag="cn")
                nc.vector.tensor_add(cn[:], m1[:], m2[:])
                tc_sb = workp.tile([P, CH, H_DIM], F16, tag="tc")
                nc.scalar.activation(tc_sb[:], cn[:], AF.Tanh)
                hn = workp.tile([P, CH, H_DIM], F16, tag="hn")
                nc.vector.tensor_mul(hn[:], sfo[:, :, 2 * H_DIM : 3 * H_DIM], tc_sb[:])

                nc.sync.dma_start(cn_out[it], cn[:].rearrange("p q h -> p (q h)"))
                nc.sync.dma_start(hn_out[it], hn[:].rearrange("p q h -> p (q h)"))

    _split_waits(nc)
    return nc


def host_prep(x, h, c, Wx, Wh, b):
    """Layout-only host prep for the full batch. Returns fp16 arrays."""
    n = x.shape[0]
    nwin = n // WIN

    A = np.empty((n, K_AUG), dtype=np.float16)
    A[:, 0:IN_DIM] = np.asarray(x, np.float32)
    A[:, IN_DIM:XH] = np.asarray(h, np.float32)
    A[:, XH] = 1.0
    # window w, chunk m, partition p  <-  row w*2048 + 16p + m
    # xh_host[w, :, m*128+p] = A[row].T
    xh_host = np.ascontiguousarray(
        A.reshape(nwin, P, CH, K_AUG).transpose(0, 3, 2, 1).reshape(nwin, K_AUG, WIN)
    )

    c_host = np.ascontiguousarray(
        np.asarray(c, np.float32).astype(np.float16).reshape(nwin, P, CH * H_DIM)
    )

    W = np.concatenate(
        [np.asarray(Wx), np.asarray(Wh), np.asarray(b)[None, :]], axis=0
    ).astype(np.float32)  # [49, 128] cols [i|f|g|o]
    W = np.concatenate(
        [W[:, 0:64], W[:, 96:128], W[:, 64:96]], axis=1
    )  # -> [i|f|o|g]
    w_host = np.ascontiguousarray(W.astype(np.float16))
    return xh_host, c_host, w_host


_NC_CACHE = {}


def _get_nc(rows=R):
    if rows not in _NC_CACHE:
        _NC_CACHE[rows] = build_nc(rows)
    return _NC_CACHE[rows]


def run(x, h, c, Wx, Wh, b, trace=False, rows=R, n_cores=N_CORES):
    """Shard, execute on the 8 cores, gather. Returns (h_new, c_new, results)."""
    from concourse.bass_utils import run_bass_kernel_spmd

    xh_host, c_host, w_host = host_prep(x, h, c, Wx, Wh, b)
    nc = _get_nc(rows)
    nwin = rows // WIN
    in_maps = []
    for i in range(n_cores):
        sl = slice(i * nwin, (i + 1) * nwin)
        in_maps.append(
            {
                "xh": xh_host[sl],
                "c_in": c_host[sl],
                "w": w_host,
            }
        )
    res = run_bass_kernel_spmd(nc, in_maps, list(range(n_cores)), trace=trace)
    n = rows * n_cores
    h_new = np.empty((n, H_DIM), dtype=np.float32)
    c_new = np.empty((n, H_DIM), dtype=np.float32)
    for i, r in enumerate(res.results):
        sl = slice(i * rows, (i + 1) * rows)
        h_new[sl] = r["hn"].reshape(rows, H_DIM).astype(np.float32)
        c_new[sl] = r["cn"].reshape(rows, H_DIM).astype(np.float32)
    return h_new, c_new, res


def kernel(x, h, c, Wx, Wh, b):
    h_new, c_new, _ = run(x, h, c, Wx, Wh, b)
    return h_new, c_new


# revision 8
# speedup vs baseline: 2.7684x; 1.3144x over previous
"""v5: row-major LSTM cell kernel, fp16 I/O, K-stacked pair matmuls.

Sharding: pure data-parallel, batch split 8 ways (131072 rows/core).

Row mapping (per core): row = w2*4096 + 32p + 16a + m, where w2 = window
pair, a = window parity, p = partition, m = chunk-in-window = 2j + half
(j = matmul pair, half = A/B slot). c/hn/cn host arrays are NATURAL
reshapes [nwin/2, 128, 2*16*32] of the row-major [R, 32] arrays.

Host prep (layout only; all compute on device):
  W_pad [128,256] fp16, cols [Ai|Af|Ao|Bi|Bf|Bo|Ag|Bg] (A rows 0:49,
    B rows 64:113 of the stationary; zeros elsewhere kill junk lanes)
  xha/xhb [nwin, 49, 8*128] fp16: window w=2*w2+a, col j*128+p =
    [x|h|1](row(w2, a, p, m=2j+half)), half=0 for xha, 1 for xhb

Device, per 2048-row window:
  - xh_sb [128, 8, 128]: parts 0:49 <- xha, 64:113 <- xhb
  - 8 matmuls: lhsT = xh_sb[:, j, :] stationary, rhs = W_pad [128, 256]
    -> PSUM [128, 8, 256] f32
  - ACT: sigmoid PSUM[:, :, 0:192] -> sfo fp16; tanh [:, :, 192:256] -> g
  - DVE (fp16 SBUF 2x): m1=i*g, m2=f*c, cn=m1+m2, hn=o*tanh(cn)
  - ACT: tc=tanh(cn)
  - c in on sync; hn/cn out on gpsimd, batched per 2 windows
"""

import sys

if "/opt/trn_rl_repo" not in sys.path:
    sys.path.insert(0, "/opt/trn_rl_repo")

import numpy as np

import bass_rust
import concourse.bass as bass
import concourse.tile as tile
from concourse import mybir

F32 = mybir.dt.float32
F16 = mybir.dt.float16
AF = mybir.ActivationFunctionType

B = 1048576
N_CORES = 8
R = B // N_CORES
IN_DIM, H_DIM = 16, 32
XH = IN_DIM + H_DIM
K_AUG = XH + 1  # 49
G4 = 4 * H_DIM  # 128
P = 128
NPAIR = 8  # matmul pairs per window
CH = 2 * NPAIR  # 16 chunks (of 128 rows) per window
WIN = CH * P  # 2048 rows per window
NWIN = R // WIN  # 64


def _split_waits(nc, max_waits=1):
    """Walrus codegen allows at most one semaphore wait per instruction."""
    n = 0
    for f in nc.m.functions:
        for blk in f.blocks:
            insts = blk.instructions
            new = []
            for inst in insts:
                si = inst.sync_info
                waits = list(si.on_wait) if si and si.on_wait else []
                if len(waits) > max_waits:
                    excess, keep = waits[:-max_waits], waits[-max_waits:]
                    for j in range(0, len(excess), max_waits):
                        nop = mybir.InstEventSemaphore(
                            name=f"{inst.name}-tw{j}", ins=[], outs=[]
                        )
                        nop.engine = inst.engine
                        nop.sync_info = bass_rust.SyncInfo(
                            on_wait=excess[j : j + max_waits], on_update=[]
                        )
                        new.append(nop)
                        n += 1
                    si.on_wait = keep
                    inst.sync_info = si
                new.append(inst)
            insts[:] = new
    return n


def build_nc(rows=R):
    assert rows % (2 * WIN) == 0
    nwin = rows // WIN

    nc = bass.Bass()
    xha = nc.dram_tensor("xha", [nwin, K_AUG, NPAIR * P], F16, kind="ExternalInput")
    xhb = nc.dram_tensor("xhb", [nwin, K_AUG, NPAIR * P], F16, kind="ExternalInput")
    c_in = nc.dram_tensor(
        "c_in", [nwin // 2, P, 2 * CH * H_DIM], F16, kind="ExternalInput"
    )
    w = nc.dram_tensor("w", [P, 2 * G4], F16, kind="ExternalInput")
    hn_out = nc.dram_tensor(
        "hn", [nwin // 2, P, 2 * CH * H_DIM], F16, kind="ExternalOutput"
    )
    cn_out = nc.dram_tensor(
        "cn", [nwin // 2, P, 2 * CH * H_DIM], F16, kind="ExternalOutput"
    )

    with tile.TileContext(nc) as tc:
        with (
            tc.tile_pool(name="const", bufs=1) as constp,
            tc.tile_pool(name="io", bufs=3) as iop,
            tc.tile_pool(name="pair", bufs=2) as pairp,
            tc.tile_pool(name="work", bufs=3) as workp,
            tc.tile_pool(name="psum", bufs=2, space="PSUM") as psump,
        ):
            w_sb = constp.tile([P, 2 * G4], F16, tag="w")
            nc.sync.dma_start(w_sb[:], w[:])

            c_t = None
            hn_t = None
            cn_t = None
            for it in range(nwin):
                half = it % 2
                xh_sb = iop.tile([P, NPAIR, P], F16, tag="xh")
                nc.sync.dma_start(
                    xh_sb[0:K_AUG], xha[it].rearrange("k (j p) -> k j p", j=NPAIR)
                )
                nc.sync.dma_start(
                    xh_sb[K_AUG : 2 * K_AUG],
                    xhb[it].rearrange("k (j p) -> k j p", j=NPAIR),
                )
                if half == 0:
                    c_t = pairp.tile([P, 2, NPAIR, 2, H_DIM], F16, tag="c")
                    nc.sync.dma_start(
                        c_t[:].rearrange("p a j m h -> p (a j m h)"), c_in[it // 2]
                    )
                    hn_t = pairp.tile([P, 2, NPAIR, 2, H_DIM], F16, tag="hn")
                    cn_t = pairp.tile([P, 2, NPAIR, 2, H_DIM], F16, tag="cn")

                ps = psump.tile([P, NPAIR, 2 * G4], F32, tag="ps")
                for j in range(NPAIR):
                    nc.tensor.matmul(
                        ps[:, j, :],
                        xh_sb[0 : 2 * K_AUG, j, :],
                        w_sb[0 : 2 * K_AUG],
                        start=True,
                        stop=True,
                    )

                # sfo free layout per pair j: [Ai|Af|Ao|Bi|Bf|Bo] (6*32)
                sfo = workp.tile([P, NPAIR, 6 * H_DIM], F16, tag="sfo")
                nc.scalar.activation(sfo[:], ps[:, :, 0 : 6 * H_DIM], AF.Sigmoid)
                g_sb = workp.tile([P, NPAIR, 2 * H_DIM], F16, tag="g")
                nc.scalar.activation(
                    g_sb[:], ps[:, :, 6 * H_DIM : 2 * G4], AF.Tanh
                )

                sfo4 = sfo[:].rearrange("p j (m g) -> p j m g", m=2)
                g4 = g_sb[:].rearrange("p j (m h) -> p j m h", m=2)
                c4 = c_t[:, half]
                m1 = workp.tile([P, NPAIR, 2, H_DIM], F16, tag="m1")
                nc.vector.tensor_mul(m1[:], sfo4[:, :, :, 0:H_DIM], g4)
                m2 = workp.tile([P, NPAIR, 2, H_DIM], F16, tag="m2")
                nc.vector.tensor_mul(m2[:], sfo4[:, :, :, H_DIM : 2 * H_DIM], c4)
                nc.vector.tensor_add(cn_t[:, half], m1[:], m2[:])
                tc_sb = workp.tile([P, NPAIR, 2, H_DIM], F16, tag="tc")
                nc.scalar.activation(tc_sb[:], cn_t[:, half], AF.Tanh)
                nc.vector.tensor_mul(
                    hn_t[:, half], sfo4[:, :, :, 2 * H_DIM : 3 * H_DIM], tc_sb[:]
                )

                if half == 1:
                    nc.gpsimd.dma_start(
                        cn_out[it // 2], cn_t[:].rearrange("p a j m h -> p (a j m h)")
                    )
                    nc.gpsimd.dma_start(
                        hn_out[it // 2], hn_t[:].rearrange("p a j m h -> p (a j m h)")
                    )

    _split_waits(nc)
    return nc


def host_prep(x, h, c, Wx, Wh, b):
    """Layout-only host prep for the full batch. Returns fp16 arrays."""
    n = x.shape[0]
    nwin = n // WIN

    A = np.empty((n, K_AUG), dtype=np.float16)
    A[:, 0:IN_DIM] = np.asarray(x, np.float32)
    A[:, IN_DIM:XH] = np.asarray(h, np.float32)
    A[:, XH] = 1.0
    # row = w2*4096 + 32p + 16a + m,  m = 2j + half
    A6 = A.reshape(nwin // 2, P, 2, NPAIR, 2, K_AUG)  # [w2, p, a, j, half, k]
    # window w = 2*w2 + a; col index = j*128 + p
    xha = np.ascontiguousarray(
        A6[:, :, :, :, 0, :].transpose(0, 2, 4, 3, 1).reshape(nwin, K_AUG, NPAIR * P)
    )
    xhb = np.ascontiguousarray(
        A6[:, :, :, :, 1, :].transpose(0, 2, 4, 3, 1).reshape(nwin, K_AUG, NPAIR * P)
    )

    c_host = np.ascontiguousarray(
        np.asarray(c, np.float32)
        .astype(np.float16)
        .reshape(nwin // 2, P, 2 * CH * H_DIM)
    )

    W = np.concatenate(
        [np.asarray(Wx), np.asarray(Wh), np.asarray(b)[None, :]], axis=0
    ).astype(np.float32)  # [49, 128] cols [i|f|g|o]
    Wi, Wf, Wg, Wo = W[:, 0:32], W[:, 32:64], W[:, 64:96], W[:, 96:128]
    Wifo = np.concatenate([Wi, Wf, Wo], axis=1)  # [49, 96]
    w_host = np.zeros((P, 2 * G4), dtype=np.float16)
    w_host[0:K_AUG, 0:96] = Wifo  # A: i f o
    w_host[K_AUG : 2 * K_AUG, 96:192] = Wifo  # B: i f o
    w_host[0:K_AUG, 192:224] = Wg  # A: g
    w_host[K_AUG : 2 * K_AUG, 224:256] = Wg  # B: g
    return xha, xhb, c_host, w_host


_NC_CACHE = {}


def _get_nc(rows=R):
    if rows not in _NC_CACHE:
        _NC_CACHE[rows] = build_nc(rows)
    return _NC_CACHE[rows]


def run(x, h, c, Wx, Wh, b, trace=False, rows=R, n_cores=N_CORES):
    """Shard, execute on the 8 cores, gather. Returns (h_new, c_new, results)."""
    from concourse.bass_utils import run_bass_kernel_spmd

    xha, xhb, c_host, w_host = host_prep(x, h, c, Wx, Wh, b)
    nc = _get_nc(rows)
    nwin = rows // WIN
    in_maps = []
    for i in range(n_cores):
        sl = slice(i * nwin, (i + 1) * nwin)
        sl2 = slice(i * nwin // 2, (i + 1) * nwin // 2)
        in_maps.append(
            {
                "xha": xha[sl],
                "xhb": xhb[sl],
                "c_in": c_host[sl2],
                "w": w_host,
            }
        )
    res = run_bass_kernel_spmd(nc, in_maps, list(range(n_cores)), trace=trace)
    n = rows * n_cores
    h_new = np.empty((n, H_DIM), dtype=np.float32)
    c_new = np.empty((n, H_DIM), dtype=np.float32)
    for i, r in enumerate(res.results):
        sl = slice(i * rows, (i + 1) * rows)
        h_new[sl] = r["hn"].reshape(rows, H_DIM).astype(np.float32)
        c_new[sl] = r["cn"].reshape(rows, H_DIM).astype(np.float32)
    return h_new, c_new, res


def kernel(x, h, c, Wx, Wh, b):
    h_new, c_new, _ = run(x, h, c, Wx, Wh, b)
    return h_new, c_new


# revision 9
# speedup vs baseline: 2.8108x; 1.0153x over previous
"""v5: row-major LSTM cell kernel, fp16 I/O, K-stacked pair matmuls.

Sharding: pure data-parallel, batch split 8 ways (131072 rows/core).

Row mapping (per core): row = w2*4096 + 32p + 16a + m, where w2 = window
pair, a = window parity, p = partition, m = chunk-in-window = 2j + half
(j = matmul pair, half = A/B slot). c/hn/cn host arrays are NATURAL
reshapes [nwin/2, 128, 2*16*32] of the row-major [R, 32] arrays.

Host prep (layout only; all compute on device):
  W_pad [128,256] fp16, cols [Ai|Af|Ao|Bi|Bf|Bo|Ag|Bg] (A rows 0:49,
    B rows 64:113 of the stationary; zeros elsewhere kill junk lanes)
  xha/xhb [nwin, 49, 8*128] fp16: window w=2*w2+a, col j*128+p =
    [x|h|1](row(w2, a, p, m=2j+half)), half=0 for xha, 1 for xhb

Device, per 2048-row window:
  - xh_sb [128, 8, 128]: parts 0:49 <- xha, 64:113 <- xhb
  - 8 matmuls: lhsT = xh_sb[:, j, :] stationary, rhs = W_pad [128, 256]
    -> PSUM [128, 8, 256] f32
  - ACT: sigmoid PSUM[:, :, 0:192] -> sfo fp16; tanh [:, :, 192:256] -> g
  - DVE (fp16 SBUF 2x): m1=i*g, m2=f*c, cn=m1+m2, hn=o*tanh(cn)
  - ACT: tc=tanh(cn)
  - c in on sync; hn/cn out on gpsimd, batched per 2 windows
"""

import sys

if "/opt/trn_rl_repo" not in sys.path:
    sys.path.insert(0, "/opt/trn_rl_repo")

import ml_dtypes
import numpy as np

import bass_rust
import concourse.bass as bass
import concourse.tile as tile
from concourse import mybir

F32 = mybir.dt.float32
F16 = mybir.dt.float16
BF16 = mybir.dt.bfloat16
AF = mybir.ActivationFunctionType

B = 1048576
N_CORES = 8
R = B // N_CORES
IN_DIM, H_DIM = 16, 32
XH = IN_DIM + H_DIM
K_AUG = XH + 1  # 49
G4 = 4 * H_DIM  # 128
P = 128
NPAIR = 8  # matmul pairs per window
CH = 2 * NPAIR  # 16 chunks (of 128 rows) per window
WIN = CH * P  # 2048 rows per window
NWIN = R // WIN  # 64


def _split_waits(nc, max_waits=1):
    """Walrus codegen allows at most one semaphore wait per instruction."""
    n = 0
    for f in nc.m.functions:
        for blk in f.blocks:
            insts = blk.instructions
            new = []
            for inst in insts:
                si = inst.sync_info
                waits = list(si.on_wait) if si and si.on_wait else []
                if len(waits) > max_waits:
                    excess, keep = waits[:-max_waits], waits[-max_waits:]
                    for j in range(0, len(excess), max_waits):
                        nop = mybir.InstEventSemaphore(
                            name=f"{inst.name}-tw{j}", ins=[], outs=[]
                        )
                        nop.engine = inst.engine
                        nop.sync_info = bass_rust.SyncInfo(
                            on_wait=excess[j : j + max_waits], on_update=[]
                        )
                        new.append(nop)
                        n += 1
                    si.on_wait = keep
                    inst.sync_info = si
                new.append(inst)
            insts[:] = new
    return n


def build_nc(rows=R):
    assert rows % (2 * WIN) == 0
    nwin = rows // WIN

    nc = bass.Bass()
    xha = nc.dram_tensor("xha", [nwin, K_AUG, NPAIR * P], BF16, kind="ExternalInput")
    xhb = nc.dram_tensor("xhb", [nwin, K_AUG, NPAIR * P], BF16, kind="ExternalInput")
    c_in = nc.dram_tensor(
        "c_in", [nwin // 2, P, 2 * CH * H_DIM], F16, kind="ExternalInput"
    )
    w = nc.dram_tensor("w", [P, 2 * G4], BF16, kind="ExternalInput")
    hn_out = nc.dram_tensor(
        "hn", [nwin // 2, P, 2 * CH * H_DIM], F16, kind="ExternalOutput"
    )
    cn_out = nc.dram_tensor(
        "cn", [nwin // 2, P, 2 * CH * H_DIM], F16, kind="ExternalOutput"
    )

    with tile.TileContext(nc) as tc:
        with (
            tc.tile_pool(name="const", bufs=1) as constp,
            tc.tile_pool(name="io", bufs=3) as iop,
            tc.tile_pool(name="pair", bufs=2) as pairp,
            tc.tile_pool(name="work", bufs=3) as workp,
            tc.tile_pool(name="psum", bufs=2, space="PSUM") as psump,
        ):
            w_sb = constp.tile([P, 2 * G4], BF16, tag="w")
            nc.sync.dma_start(w_sb[:], w[:])

            c_t = None
            hn_t = None
            cn_t = None
            for it in range(nwin):
                half = it % 2
                xh_sb = iop.tile([P, NPAIR, P], BF16, tag="xh")
                nc.sync.dma_start(
                    xh_sb[0:K_AUG], xha[it].rearrange("k (j p) -> k j p", j=NPAIR)
                )
                nc.sync.dma_start(
                    xh_sb[K_AUG : 2 * K_AUG],
                    xhb[it].rearrange("k (j p) -> k j p", j=NPAIR),
                )
                if half == 0:
                    c_t = pairp.tile([P, 2, NPAIR, 2, H_DIM], F16, tag="c")
                    nc.sync.dma_start(
                        c_t[:].rearrange("p a j m h -> p (a j m h)"), c_in[it // 2]
                    )
                    hn_t = pairp.tile([P, 2, NPAIR, 2, H_DIM], F16, tag="hn")
                    cn_t = pairp.tile([P, 2, NPAIR, 2, H_DIM], F16, tag="cn")

                ps = psump.tile([P, NPAIR, 2 * G4], F32, tag="ps")
                for j in range(NPAIR):
                    nc.tensor.matmul(
                        ps[:, j, :],
                        xh_sb[0 : 2 * K_AUG, j, :],
                        w_sb[0 : 2 * K_AUG],
                        start=True,
                        stop=True,
                    )

                # sfo free layout per pair j: [Ai|Af|Ao|Bi|Bf|Bo] (6*32)
                sfo = workp.tile([P, NPAIR, 6 * H_DIM], F16, tag="sfo")
                nc.scalar.activation(sfo[:], ps[:, :, 0 : 6 * H_DIM], AF.Sigmoid)
                g_sb = workp.tile([P, NPAIR, 2 * H_DIM], F16, tag="g")
                nc.scalar.activation(
                    g_sb[:], ps[:, :, 6 * H_DIM : 2 * G4], AF.Tanh
                )

                sfo4 = sfo[:].rearrange("p j (m g) -> p j m g", m=2)
                g4 = g_sb[:].rearrange("p j (m h) -> p j m h", m=2)
                c4 = c_t[:, half]
                m1 = workp.tile([P, NPAIR, 2, H_DIM], F16, tag="m1")
                nc.vector.tensor_mul(m1[:], sfo4[:, :, :, 0:H_DIM], g4)
                m2 = workp.tile([P, NPAIR, 2, H_DIM], F16, tag="m2")
                nc.vector.tensor_mul(m2[:], sfo4[:, :, :, H_DIM : 2 * H_DIM], c4)
                nc.vector.tensor_add(cn_t[:, half], m1[:], m2[:])
                tc_sb = workp.tile([P, NPAIR, 2, H_DIM], F16, tag="tc")
                nc.scalar.activation(tc_sb[:], cn_t[:, half], AF.Tanh)
                nc.vector.tensor_mul(
                    hn_t[:, half], sfo4[:, :, :, 2 * H_DIM : 3 * H_DIM], tc_sb[:]
                )

                if half == 1:
                    nc.gpsimd.dma_start(
                        cn_out[it // 2], cn_t[:].rearrange("p a j m h -> p (a j m h)")
                    )
                    nc.gpsimd.dma_start(
                        hn_out[it // 2], hn_t[:].rearrange("p a j m h -> p (a j m h)")
                    )

    _split_waits(nc)
    return nc


def host_prep(x, h, c, Wx, Wh, b):
    """Layout-only host prep for the full batch. Returns fp16 arrays."""
    n = x.shape[0]
    nwin = n // WIN

    A = np.empty((n, K_AUG), dtype=ml_dtypes.bfloat16)
    A[:, 0:IN_DIM] = np.asarray(x, np.float32)
    A[:, IN_DIM:XH] = np.asarray(h, np.float32)
    A[:, XH] = 1.0
    # row = w2*4096 + 32p + 16a + m,  m = 2j + half
    A6 = A.reshape(nwin // 2, P, 2, NPAIR, 2, K_AUG)  # [w2, p, a, j, half, k]
    # window w = 2*w2 + a; col index = j*128 + p
    xha = np.ascontiguousarray(
        A6[:, :, :, :, 0, :].transpose(0, 2, 4, 3, 1).reshape(nwin, K_AUG, NPAIR * P)
    )
    xhb = np.ascontiguousarray(
        A6[:, :, :, :, 1, :].transpose(0, 2, 4, 3, 1).reshape(nwin, K_AUG, NPAIR * P)
    )

    c_host = np.ascontiguousarray(
        np.asarray(c, np.float32)
        .astype(np.float16)
        .reshape(nwin // 2, P, 2 * CH * H_DIM)
    )

    W = np.concatenate(
        [np.asarray(Wx), np.asarray(Wh), np.asarray(b)[None, :]], axis=0
    ).astype(np.float32)  # [49, 128] cols [i|f|g|o]
    Wi, Wf, Wg, Wo = W[:, 0:32], W[:, 32:64], W[:, 64:96], W[:, 96:128]
    Wifo = np.concatenate([Wi, Wf, Wo], axis=1)  # [49, 96]
    w_host = np.zeros((P, 2 * G4), dtype=ml_dtypes.bfloat16)
    w_host[0:K_AUG, 0:96] = Wifo  # A: i f o
    w_host[K_AUG : 2 * K_AUG, 96:192] = Wifo  # B: i f o
    w_host[0:K_AUG, 192:224] = Wg  # A: g
    w_host[K_AUG : 2 * K_AUG, 224:256] = Wg  # B: g
    return xha, xhb, c_host, w_host


_NC_CACHE = {}


def _get_nc(rows=R):
    if rows not in _NC_CACHE:
        _NC_CACHE[rows] = build_nc(rows)
    return _NC_CACHE[rows]


def run(x, h, c, Wx, Wh, b, trace=False, rows=R, n_cores=N_CORES):
    """Shard, execute on the 8 cores, gather. Returns (h_new, c_new, results)."""
    from concourse.bass_utils import run_bass_kernel_spmd

    xha, xhb, c_host, w_host = host_prep(x, h, c, Wx, Wh, b)
    nc = _get_nc(rows)
    nwin = rows // WIN
    in_maps = []
    for i in range(n_cores):
        sl = slice(i * nwin, (i + 1) * nwin)
        sl2 = slice(i * nwin // 2, (i + 1) * nwin // 2)
        in_maps.append(
            {
                "xha": xha[sl],
                "xhb": xhb[sl],
                "c_in": c_host[sl2],
                "w": w_host,
            }
        )
    res = run_bass_kernel_spmd(nc, in_maps, list(range(n_cores)), trace=trace)
    n = rows * n_cores
    h_new = np.empty((n, H_DIM), dtype=np.float32)
    c_new = np.empty((n, H_DIM), dtype=np.float32)
    for i, r in enumerate(res.results):
        sl = slice(i * rows, (i + 1) * rows)
        h_new[sl] = r["hn"].reshape(rows, H_DIM).astype(np.float32)
        c_new[sl] = r["cn"].reshape(rows, H_DIM).astype(np.float32)
    return h_new, c_new, res


def kernel(x, h, c, Wx, Wh, b):
    h_new, c_new, _ = run(x, h, c, Wx, Wh, b)
    return h_new, c_new
